# revision 1
# baseline (speedup 1.0000x reference)
"""Trainium2 Bass kernel for nn_CPA_43 (dense transformer block, CPA attention).

Data-parallel over batch: B=256 sharded as 32 samples per core across 8 cores.
All weights replicated. Two on-chip stages per core:
  stage 1: LN1/LN2, Q/K/V projections, channel-softmax(q), position-softmax(k),
           context/attention matmuls, Wr + residual -> f3out (spilled to DRAM),
           LN3 statistics on the fly (bn_stats).
  stage 2: LN3 apply, MLP (W1 -> gelu -> W2) + residual -> output.

Projection/MLP matmuls run in float32r (full PE rate at free dim >= 256,
~1.5e-4 rel err). The attention inner path (softmaxed q/k, v, context) runs in
bf16 — full PE rate at any free dim. Per-channel / per-position biases are
preloaded into PSUM with identity/ones matmuls; LN gains and the positional
projections are folded into weights/biases on the host.

Activation-table sets are pinned to natural_log_exp_and_others (stage 1) and
gelu_and_others (stage 2) to avoid ~2.7us table reloads.
"""

import numpy as np

B, N3, N4, DIM, HEADS, MLP_DIM = 256, 256, 64, 512, 8, 2048
N_CORES = 8
BSH = B // N_CORES  # samples per core
EPS = 1e-5
SG4 = 8  # f4-group size (samples per K/V block)

_BUILD_CACHE = {}


def _host_prep(inputs):
    """Fold LN gains + positional projections into weights/biases (exact)."""
    f = {k: np.asarray(v, dtype=np.float64) for k, v in inputs.items()}
    pos3 = f["pos3"][0]  # [N3, DIM]
    pos4 = f["pos4"][0]  # [N4, DIM]

    import ml_dtypes
    bf16 = ml_dtypes.bfloat16
    wq = np.ascontiguousarray((f["ln1_g"][:, None] * f["Wq"]).astype(bf16))
    wk = np.ascontiguousarray((f["ln2_g"][:, None] * f["Wk"]).astype(bf16))
    wv = np.ascontiguousarray((f["ln2_g"][:, None] * f["Wv"]).astype(bf16))
    wr = np.ascontiguousarray(f["Wr"].astype(np.float32))
    w1 = np.ascontiguousarray((f["ln3_g"][:, None] * f["W1"]).astype(bf16))
    w2 = np.ascontiguousarray(f["W2"].astype(bf16))

    biasq = ((f["ln1_b"][None, :] + pos3) @ f["Wq"] + f["bq"]).astype(np.float32)  # [N3, DIM]
    biask = ((f["ln2_b"][None, :] + pos4) @ f["Wk"] + f["bk"]).astype(np.float32)  # [N4, DIM]
    biasv = (f["ln2_b"] @ f["Wv"] + f["bv"]).astype(np.float32)  # [DIM]
    bias1 = (f["ln3_b"] @ f["W1"] + f["b1"]).astype(np.float32)  # [MLP]
    br = f["br"].astype(np.float32)
    b2 = f["b2"].astype(np.float32)

    # biask in channel-major, tiled over the SG4 samples of an f4-group:
    # [DIM, SG4*N4] with column order (sample_in_group, position)
    biask_cm = np.tile(biask.T[:, None, :], (1, SG4, 1)).reshape(DIM, SG4 * N4)
    biask_cm = np.ascontiguousarray(biask_cm.astype(np.float32))
    # bias1 as [128, 16]: column hc holds biases for hidden channels hc*128..+128
    bias1_cm = np.ascontiguousarray(bias1.reshape(MLP_DIM // 128, 128).T.astype(np.float32))

    return {
        "wq": wq, "wk": wk, "wv": wv, "wr": wr, "w1": w1, "w2": w2,
        "biasq": np.ascontiguousarray(biasq),
        "biask_cm": biask_cm,
        "biasv_row": np.ascontiguousarray(biasv[None, :]),
        "br_row": np.ascontiguousarray(br[None, :]),
        "b2_row": np.ascontiguousarray(b2[None, :]),
        "bias1_cm": bias1_cm,
        "ones_col": np.ones((1, 128), dtype=np.float32),
        "ident": np.eye(128, dtype=np.float32),
        "ident_bf": np.eye(128, dtype=np.float32),  # cast to bf16 tile on chip
    }


def _build(n_samples, repeat=1):
    """Build the Bacc module for one core processing `n_samples` samples.

    `repeat` re-runs the whole computation that many times back-to-back —
    used only for wall-clock timing amplification in test.py."""
    import concourse.bacc as bacc
    import concourse.tile as tile
    import concourse.mybir as mybir
    from concourse.bass import AP  # noqa: F401

    # Restrict activation-table-set choices (see module docstring).
    if not hasattr(bacc, "_orig_get_activation_tables"):
        bacc._orig_get_activation_tables = bacc.get_activation_tables

        def _gat(arch):
            full = bacc._orig_get_activation_tables(arch)
            keep = {"natural_log_exp_and_others", "gelu_and_others"}
            return {n: (s if n in keep else set()) for n, s in full.items()}

        bacc.get_activation_tables = _gat

    F32 = mybir.dt.float32
    F32R = mybir.dt.float32r
    BF16 = mybir.dt.bfloat16
    AX = mybir.AxisListType.X
    ALU = mybir.AluOpType
    ACTF = mybir.ActivationFunctionType

    NS = n_samples
    assert NS % SG4 == 0
    NG4 = NS // SG4      # f4 groups
    NG2 = NS // 2        # mlp groups of 2 samples

    nc = bacc.Bacc("TRN2", debug=False, num_devices=N_CORES)

    f3 = nc.dram_tensor("f3", [NS, N3, DIM], F32, kind="ExternalInput").ap()
    f4 = nc.dram_tensor("f4", [NS, N4, DIM], F32, kind="ExternalInput").ap()
    wq = nc.dram_tensor("wq", [DIM, DIM], BF16, kind="ExternalInput").ap()
    wk = nc.dram_tensor("wk", [DIM, DIM], BF16, kind="ExternalInput").ap()
    wv = nc.dram_tensor("wv", [DIM, DIM], BF16, kind="ExternalInput").ap()
    wr = nc.dram_tensor("wr", [DIM, DIM], F32R, kind="ExternalInput").ap()
    w1 = nc.dram_tensor("w1", [DIM, MLP_DIM], BF16, kind="ExternalInput").ap()
    w2 = nc.dram_tensor("w2", [MLP_DIM, DIM], BF16, kind="ExternalInput").ap()
    biasq = nc.dram_tensor("biasq", [N3, DIM], F32R, kind="ExternalInput").ap()
    biask_cm = nc.dram_tensor("biask_cm", [DIM, SG4 * N4], F32R, kind="ExternalInput").ap()
    biasv_row = nc.dram_tensor("biasv_row", [1, DIM], F32R, kind="ExternalInput").ap()
    br_row = nc.dram_tensor("br_row", [1, DIM], F32R, kind="ExternalInput").ap()
    b2_row = nc.dram_tensor("b2_row", [1, DIM], F32R, kind="ExternalInput").ap()
    bias1_cm = nc.dram_tensor("bias1_cm", [128, MLP_DIM // 128], F32, kind="ExternalInput").ap()
    ones_col = nc.dram_tensor("ones_col", [1, 128], F32R, kind="ExternalInput").ap()
    ident = nc.dram_tensor("ident", [128, 128], F32R, kind="ExternalInput").ap()
    ident_bf = nc.dram_tensor("ident_bf", [128, 128], F32, kind="ExternalInput").ap()
    out = nc.dram_tensor("out", [NS, N3, DIM], F32, kind="ExternalOutput").ap()

    with tile.TileContext(nc) as tc:
        # ---- pools alive for the whole kernel ----
        with (
            tc.tile_pool(name="consts", bufs=1) as cpool,
            tc.tile_pool(name="wattn", bufs=1) as wpool,
            tc.tile_pool(name="stats", bufs=1) as spool,
            tc.tile_pool(name="dram", bufs=1, space="DRAM") as dpool,
        ):
            ident_sb = cpool.tile([128, 128], F32R, tag="ident")
            nc.sync.dma_start(ident_sb[:], ident)
            identf_sb = cpool.tile([128, 128], F32, tag="identf")
            nc.sync.dma_start(identf_sb[:], ident_bf)
            identb_sb = cpool.tile([128, 128], BF16, tag="identb")
            nc.vector.tensor_copy(identb_sb[:], identf_sb[:])
            eps_sb = cpool.tile([128, 1], F32, tag="eps")
            nc.vector.memset(eps_sb[:], EPS)
            ones_sb = cpool.tile([1, 128], F32R, tag="ones")
            nc.sync.dma_start(ones_sb[:], ones_col)
            bvrow_sb = cpool.tile([1, DIM], F32R, tag="bvrow")
            nc.sync.dma_start(bvrow_sb[:], biasv_row)
            brrow_sb = cpool.tile([1, DIM], F32R, tag="brrow")
            b2row_sb = cpool.tile([1, DIM], F32R, tag="b2row")
            bq_sb = cpool.tile([128, 2, DIM], F32R, tag="bq")
            bk_sb = cpool.tile([128, 4, SG4 * N4], F32R, tag="bk")
            nc.sync.dma_start(bk_sb[:], biask_cm.rearrange("(c p) d -> p c d", p=128))
            b1_sb = cpool.tile([128, MLP_DIM // 128], F32, tag="b1")
            # persistent block-diagonal context tiles (off-diagonal stays zero)
            ctxbd = cpool.tile([128, 4, 128], BF16, tag="ctxbd")
            nc.vector.memset(ctxbd[:], 0.0)

            wq_sb = wpool.tile([128, 4, DIM], BF16, tag="wq")
            wk_sb = wpool.tile([128, 4, DIM], BF16, tag="wk")
            nc.sync.dma_start(wk_sb[:], wk.rearrange("(c p) d -> p c d", p=128))
            wv_sb = wpool.tile([128, 4, DIM], BF16, tag="wv")
            nc.sync.dma_start(wv_sb[:], wv.rearrange("(c p) d -> p c d", p=128))
            wr_sb = wpool.tile([128, 4, DIM], F32R, tag="wr")
            w1_sb = wpool.tile([128, 4, MLP_DIM], BF16, tag="w1")
            w2_sb = wpool.tile([128, 16, DIM], BF16, tag="w2")

            # LN3 (mean, var) per token-chunk column, filled during stage 1
            stats3 = spool.tile([128, 2 * NS, 2], F32, tag="stats3")

            f3o_dram = dpool.tile([NS, N3, DIM], F32, tag="f3spill")

            for _rep in range(repeat):
                # ================= STAGE 1 =================
                with (
                    tc.tile_pool(name="s1_sb", bufs=2) as p1,
                    tc.tile_pool(name="s1_sb3", bufs=3) as p13,
                    tc.tile_pool(name="kv", bufs=3) as pkv,
                    tc.tile_pool(name="ps_tp", bufs=2, space="PSUM") as ps_tp,
                    tc.tile_pool(name="ps_mm", bufs=1, space="PSUM") as ps_mm,
                    tc.tile_pool(name="ps_ctx", bufs=1, space="PSUM") as ps_ctx,
                    tc.tile_pool(name="ps_att", bufs=1, space="PSUM") as ps_att,
                ):
                    def inv_std_from_var(var_view, sinv_view, n_cols, tag):
                        """sinv = exp(-0.5*ln(var+eps)) on [128, n_cols] views."""
                        lnv = p1.tile([128, n_cols], F32, tag=f"lnv_{tag}")
                        nc.scalar.activation(lnv[:], var_view, ACTF.Ln, bias=eps_sb[:])
                        nc.scalar.activation(sinv_view, lnv[:], ACTF.Exp, scale=-0.5)

                    def f4_block(g):
                            # ---------- f4 block: SG4 samples ----------
                            ntb = SG4 // 2  # token-chunks of 128 (2 samples each)
                            x4 = p1.tile([128, ntb, DIM], F32, tag="x4")
                            for t in range(ntb):
                                nc.sync.dma_start(
                                    x4[:, t, :],
                                    f4[SG4 * g + 2 * t: SG4 * g + 2 * t + 2].rearrange(
                                        "a b d -> (a b) d"
                                    ),
                                )
                            mv4 = p1.tile([128, ntb, 2], F32, tag="mv4")
                            for t in range(ntb):
                                bns = p1.tile([128, 6], F32, tag="bns4")
                                nc.vector.bn_stats(bns[:], x4[:, t, :])
                                nc.vector.bn_aggr(mv4[:, t, :], bns[:])
                            negm4 = p1.tile([128, ntb], F32, tag="negm4")
                            nc.vector.tensor_scalar_mul(negm4[:], mv4[:, :, 0], -1.0)
                            sinv4 = p1.tile([128, ntb], F32, tag="sinv4")
                            inv_std_from_var(mv4[:, :, 1], sinv4[:], ntb, "s4")
                            x4h = p1.tile([128, ntb, DIM], BF16, tag="x4h")
                            for t in range(ntb):
                                nc.vector.tensor_scalar(
                                    x4h[:, t, :], x4[:, t, :],
                                    negm4[:, t: t + 1], sinv4[:, t: t + 1],
                                    op0=ALU.add, op1=ALU.mult,
                                )
                            # transpose to channel-major [512ch, SG4*64 tok]
                            x4c = p1.tile([128, 4, SG4 * N4], BF16, tag="x4c", bufs=3)
                            for cc in range(4):
                                ptpf = ps_tp.tile([128, 512], F32R, tag="tp", name="ptpf")
                                ptp = ptpf.bitcast(BF16)
                                for t in range(ntb):
                                    nc.tensor.transpose(
                                        ptp[:, t * 128:(t + 1) * 128],
                                        x4h[:, t, cc * 128:(cc + 1) * 128],
                                        identb_sb[:],
                                    )
                                nc.scalar.activation(x4c[:, cc, :], ptp[:, 0:512], ACTF.Copy)
                            # K projection (channel-major out) + bias preload + exp
                            kx = p1.tile([128, 4, SG4 * N4], BF16, tag="kx")
                            ks = p1.tile([128, 4 * SG4], F32, tag="ks")
                            for cc in range(4):
                                pk = ps_mm.tile([128, 512], F32, tag="mmk", bufs=1)
                                nc.tensor.matmul(
                                    pk[:], ident_sb[:], bk_sb[:, cc, :],
                                    start=True, stop=False,
                                )
                                for kc in range(4):
                                    nc.tensor.matmul(
                                        pk[:],
                                        wk_sb[:, kc, cc * 128:(cc + 1) * 128],
                                        x4c[:, kc, :],
                                        start=False, stop=(kc == 3),
                                    )
                                nc.scalar.activation(kx[:, cc, :], pk[:], ACTF.Exp)
                                nc.vector.reduce_sum(
                                    ks[:, cc * SG4:(cc + 1) * SG4],
                                    kx[:, cc, :].rearrange("p (s d) -> p s d", s=SG4),
                                    axis=AX,
                                )
                            kr = pkv.tile([128, 4 * SG4], F32, tag="kr")
                            nc.vector.reciprocal(kr[:], ks[:])
                            # V projection (token-major out) + bias preload
                            v_tm = pkv.tile([128, ntb, DIM], BF16, tag="v_tm")
                            for t in range(ntb):
                                pv = ps_mm.tile([128, 512], F32, tag="mmk", bufs=1)
                                nc.tensor.matmul(pv[:], ones_sb[:], bvrow_sb[:], start=True, stop=False)
                                for kc in range(4):
                                    nc.tensor.matmul(
                                        pv[:],
                                        x4c[:, kc, t * 128:(t + 1) * 128],
                                        wv_sb[:, kc, :],
                                        start=False, stop=(kc == 3),
                                    )
                                nc.scalar.activation(v_tm[:, t, :], pv[:], ACTF.Copy)
                            # k back to token-major (bf16)
                            k_tm = pkv.tile([128, ntb, DIM], BF16, tag="k_tm")
                            for t in range(ntb):
                                ptp2f = ps_tp.tile([128, 512], F32R, tag="tp", name="ptp2f")
                                ptp2 = ptp2f.bitcast(BF16)
                                for cc in range(4):
                                    nc.tensor.transpose(
                                        ptp2[:, cc * 128:(cc + 1) * 128],
                                        kx[:, cc, t * 128:(t + 1) * 128],
                                        identb_sb[:],
                                    )
                                nc.vector.tensor_copy(k_tm[:, t, :], ptp2[:, 0:512])

                            return k_tm, v_tm, kr

                    def x3_load(s):
                        x3 = p13.tile([128, 2, DIM], F32, tag="x3", name="x3")
                        for t in range(2):
                            nc.sync.dma_start(
                                x3[:, t, :], f3[s, t * 128:(t + 1) * 128, :]
                            )
                        return x3

                    x3state = {}
                    x3state[0] = x3_load(0)
                    x3state[1] = x3_load(1)
                    kvstate = {}
                    kvstate[0] = f4_block(0)
                    # deferred loads: first needed ~8-12us in (Q/Wr of sample 0)
                    nc.sync.dma_start(bq_sb[:], biasq.rearrange("(t p) d -> p t d", p=128))
                    nc.sync.dma_start(wq_sb[:], wq.rearrange("(c p) d -> p c d", p=128))
                    nc.sync.dma_start(wr_sb[:], wr.rearrange("(c p) d -> p c d", p=128))
                    nc.sync.dma_start(brrow_sb[:], br_row)
                    if NG4 > 1:
                        kvstate[1] = f4_block(1)
                    # W1/W2 loads deferred here so startup DMA bandwidth goes
                    # to the first groups' activations and attention weights.
                    nc.sync.dma_start(w1_sb[:], w1.rearrange("(c p) d -> p c d", p=128))
                    nc.sync.dma_start(w2_sb[:], w2.rearrange("(c p) d -> p c d", p=128))
                    nc.sync.dma_start(b1_sb[:], bias1_cm)
                    nc.sync.dma_start(b2row_sb[:], b2_row)
                    for g in range(NG4):
                        k_tm, v_tm, kr = kvstate.pop(g)
                        # ---------- f3 blocks: SG4 samples ----------
                        for si in range(SG4):
                            if si == 4 and g + 2 < NG4:
                                kvstate[g + 2] = f4_block(g + 2)
                            s = SG4 * g + si
                            tb = si // 2
                            pb = (si % 2) * 64
                            x3 = x3state.pop(s)
                            if s + 2 < NS:
                                x3state[s + 2] = x3_load(s + 2)
                            mv1 = p1.tile([128, 2, 2], F32, tag="mv1")
                            for t in range(2):
                                bns1 = p1.tile([128, 6], F32, tag="bns1")
                                nc.vector.bn_stats(bns1[:], x3[:, t, :])
                                nc.vector.bn_aggr(mv1[:, t, :], bns1[:])
                            negm1 = p1.tile([128, 2], F32, tag="negm1")
                            nc.vector.tensor_scalar_mul(negm1[:], mv1[:, :, 0], -1.0)
                            sinv1 = p1.tile([128, 2], F32, tag="sinv1")
                            inv_std_from_var(mv1[:, :, 1], sinv1[:], 2, "s1")
                            x3h = p1.tile([128, 2, DIM], BF16, tag="x3h", bufs=3)
                            for t in range(2):
                                nc.vector.tensor_scalar(
                                    x3h[:, t, :], x3[:, t, :],
                                    negm1[:, t: t + 1], sinv1[:, t: t + 1],
                                    op0=ALU.add, op1=ALU.mult,
                                )
                            x3c = p1.tile([128, 4, 256], BF16, tag="x3c", bufs=4)
                            for cc in range(4):
                                ptpf3 = ps_tp.tile([128, 512], F32R, tag="tp", name="ptpf3")
                                ptp = ptpf3.bitcast(BF16)
                                for t in range(2):
                                    nc.tensor.transpose(
                                        ptp[:, t * 128:(t + 1) * 128],
                                        x3h[:, t, cc * 128:(cc + 1) * 128],
                                        identb_sb[:],
                                    )
                                nc.vector.tensor_copy(x3c[:, cc, :], ptp[:, 0:256])
                            # Q projection + biasq preload, then exp over both chunks
                            e_tm = p1.tile([128, 2, DIM], BF16, tag="e_tm", bufs=3)
                            for t in range(2):
                                pq = ps_mm.tile([128, 512], F32, tag="mmq", name="pq", bufs=3)
                                nc.tensor.matmul(
                                    pq[:], ident_sb[:], bq_sb[:, t, :],
                                    start=True, stop=False,
                                )
                                for kc in range(4):
                                    nc.tensor.matmul(
                                        pq[:],
                                        x3c[:, kc, t * 128:(t + 1) * 128],
                                        wq_sb[:, kc, :],
                                        start=False, stop=(kc == 3),
                                    )
                                nc.scalar.activation(e_tm[:, t, :], pq[:], ACTF.Exp)
                            qs = p1.tile([128, 16], F32, tag="qs")
                            nc.vector.reduce_sum(
                                qs[:],
                                e_tm.rearrange("p a (h d) -> p (a h) d", h=8)[:],
                                axis=AX,
                            )
                            qr = p1.tile([128, 16], F32, tag="qr")
                            nc.vector.reciprocal(qr[:], qs[:])
                            q_tm = p1.tile([128, 2, DIM], BF16, tag="q_tm", bufs=3)
                            nc.vector.tensor_tensor(
                                q_tm.rearrange("p a (h d) -> p (a h) d", h=8)[:],
                                e_tm.rearrange("p a (h d) -> p (a h) d", h=8)[:],
                                qr[:].unsqueeze(-1).broadcast_to([128, 16, 64]),
                                op=ALU.mult,
                            )
                            q_cm = p1.tile([128, 4, 256], BF16, tag="q_cm", bufs=4)
                            for cc in range(4):
                                ptpbf = ps_tp.tile([128, 512], F32R, tag="tp", name="ptpbf")
                                ptpb = ptpbf.bitcast(BF16)
                                for t in range(2):
                                    nc.tensor.transpose(
                                        ptpb[:, t * 128:(t + 1) * 128],
                                        q_tm[:, t, cc * 128:(cc + 1) * 128],
                                        identb_sb[:],
                                    )
                                nc.vector.tensor_copy(q_cm[:, cc, :], ptpb[:, 0:256])
                            # attention per head-pair
                            att_cm = p1.tile([128, 4, 256], F32R, tag="att_cm")
                            for hp in range(4):
                                pctx = ps_ctx.tile([128, 128], F32, tag="ctx")
                                nc.tensor.matmul(
                                    pctx[:],
                                    k_tm[pb:pb + 64, tb, hp * 128:(hp + 1) * 128],
                                    v_tm[pb:pb + 64, tb, hp * 128:(hp + 1) * 128],
                                    start=True, stop=True,
                                )
                                for hh in range(2):
                                    nc.vector.tensor_scalar_mul(
                                        ctxbd[hh * 64:(hh + 1) * 64, hp, hh * 64:(hh + 1) * 64],
                                        pctx[hh * 64:(hh + 1) * 64, hh * 64:(hh + 1) * 64],
                                        kr[hh * 64:(hh + 1) * 64, hp * SG4 + si: hp * SG4 + si + 1],
                                    )
                                patt = ps_att.tile([128, 256], F32, tag="att")
                                nc.tensor.matmul(
                                    patt[:], ctxbd[:, hp, :], q_cm[:, hp, :],
                                    start=True, stop=True,
                                )
                                nc.scalar.activation(att_cm[:, hp, :], patt[:], ACTF.Copy)
                            # Wr + residual -> f3out (+ LN3 stats via bn_stats)
                            for t in range(2):
                                po = ps_mm.tile([128, 512], F32, tag="mmq", bufs=3)
                                nc.tensor.matmul(po[:], ones_sb[:], brrow_sb[:], start=True, stop=False)
                                for cc in range(4):
                                    nc.tensor.matmul(
                                        po[:],
                                        att_cm[:, cc, t * 128:(t + 1) * 128],
                                        wr_sb[:, cc, :],
                                        start=False, stop=(cc == 3),
                                    )
                                f3o = p13.tile([128, DIM], F32, tag="f3o")
                                nc.vector.scalar_tensor_tensor(
                                    f3o[:], po[:], 1.0, x3[:, t, :],
                                    op0=ALU.mult, op1=ALU.add,
                                )
                                bns3 = p1.tile([128, 6], F32, tag="bns3")
                                nc.vector.bn_stats(bns3[:], f3o[:])
                                nc.vector.bn_aggr(stats3[:, 2 * s + t, :], bns3[:])
                                nc.sync.dma_start(
                                    f3o_dram[s, t * 128:(t + 1) * 128, :], f3o[:]
                                )

                # ================= STAGE 2 =================
                with (
                    tc.tile_pool(name="s2_sb", bufs=3) as p2,
                    tc.tile_pool(name="s2_sb3", bufs=3) as p23,
                    tc.tile_pool(name="ps2_tp", bufs=2, space="PSUM") as ps2_tp,
                    tc.tile_pool(name="ps2_w1", bufs=2, space="PSUM") as ps2_w1,
                    tc.tile_pool(name="ps2_w2", bufs=4, space="PSUM") as ps2_w2,
                ):
                    # LN3 stats math for all samples at once
                    negm3 = p2.tile([128, 2 * NS], F32, tag="negm3")
                    nc.vector.tensor_scalar_mul(negm3[:], stats3[:, :, 0], -1.0)
                    lnv3 = p2.tile([128, 2 * NS], F32, tag="lnv3")
                    nc.scalar.activation(lnv3[:], stats3[:, :, 1], ACTF.Ln, bias=eps_sb[:])
                    s3 = p2.tile([128, 2 * NS], F32, tag="s3")
                    nc.scalar.activation(s3[:], lnv3[:], ACTF.Exp, scale=-0.5)

                    for g in range(NG2):
                        f3o2 = p23.tile([128, 4, DIM], F32, tag="f3o2")
                        for c in range(4):
                            nc.sync.dma_start(
                                f3o2[:, c, :],
                                f3o_dram[2 * g + c // 2, (c % 2) * 128:(c % 2) * 128 + 128, :],
                            )
                        xoh = p2.tile([128, 4, DIM], BF16, tag="xoh")
                        for c in range(4):
                            col = 4 * g + c
                            nc.vector.tensor_scalar(
                                xoh[:, c, :], f3o2[:, c, :],
                                negm3[:, col: col + 1], s3[:, col: col + 1],
                                op0=ALU.add, op1=ALU.mult,
                            )
                        xoc = p2.tile([128, 4, DIM], BF16, tag="xoc")
                        for cc in range(4):
                            ptpf2 = ps2_tp.tile([128, 512], F32R, tag="tp2", name="ptpf2")
                            ptp = ptpf2.bitcast(BF16)
                            for c in range(4):
                                nc.tensor.transpose(
                                    ptp[:, c * 128:(c + 1) * 128],
                                    xoh[:, c, cc * 128:(cc + 1) * 128],
                                    identb_sb[:],
                                )
                            nc.scalar.activation(xoc[:, cc, :], ptp[:, 0:512], ACTF.Copy)
                        pf = []
                        for c in range(4):
                            pfc = ps2_w2.tile([128, 512], F32, tag="w2acc")
                            nc.tensor.matmul(pfc[:], ones_sb[:], b2row_sb[:], start=True, stop=False)
                            pf.append(pfc)
                        for hc in range(16):
                            pw1 = ps2_w1.tile([128, 512], F32, tag="w1ps")
                            for kc in range(4):
                                nc.tensor.matmul(
                                    pw1[:],
                                    w1_sb[:, kc, hc * 128:(hc + 1) * 128],
                                    xoc[:, kc, :],
                                    start=(kc == 0), stop=(kc == 3),
                                )
                            gt = p23.tile([128, DIM], BF16, tag="gt")
                            nc.scalar.activation(
                                gt[:], pw1[:], ACTF.Gelu, bias=b1_sb[:, hc: hc + 1]
                            )
                            for c in range(4):
                                nc.tensor.matmul(
                                    pf[c][:],
                                    gt[:, c * 128:(c + 1) * 128],
                                    w2_sb[:, hc, :],
                                    start=False, stop=(hc == 15),
                                    skip_group_check=True,
                                )
                        for c in range(4):
                            outt = p2.tile([128, DIM], F32, tag="outt")
                            nc.vector.tensor_add(outt[:], pf[c][:], f3o2[:, c, :])
                            nc.sync.dma_start(
                                out[2 * g + c // 2, (c % 2) * 128:(c % 2) * 128 + 128, :],
                                outt[:],
                            )

    nc.compile()
    return nc


def _get_module(n_samples):
    if n_samples not in _BUILD_CACHE:
        _BUILD_CACHE[n_samples] = _build(n_samples)
    return _BUILD_CACHE[n_samples]


def kernel(**inputs) -> np.ndarray:
    from concourse.bass_utils import run_bass_kernel_spmd

    consts = _host_prep(inputs)
    f3 = np.ascontiguousarray(np.asarray(inputs["f3"], dtype=np.float32))
    f4 = np.ascontiguousarray(np.asarray(inputs["f4"], dtype=np.float32))

    nc = _get_module(BSH)
    in_maps = []
    for c in range(N_CORES):
        m = dict(consts)
        m["f3"] = np.ascontiguousarray(f3[c * BSH:(c + 1) * BSH])
        m["f4"] = np.ascontiguousarray(f4[c * BSH:(c + 1) * BSH])
        in_maps.append(m)
    res = run_bass_kernel_spmd(nc, in_maps, core_ids=list(range(N_CORES)))
    return np.concatenate([res.results[c]["out"] for c in range(N_CORES)], axis=0)



# revision 13
# speedup vs baseline: 2.9140x; 2.9140x over previous
"""Trainium2 Bass kernel for nn_CPA_43 (dense transformer block, CPA attention).

Data-parallel over batch: B=256 sharded as 32 samples per core across 8 cores.
All weights replicated. Two on-chip stages per core:
  stage 1: LN1/LN2, Q/K/V projections, channel-softmax(q), position-softmax(k),
           context/attention matmuls, Wr + residual -> f3out (spilled to DRAM),
           LN3 statistics on the fly (bn_stats).
  stage 2: LN3 apply, MLP (W1 -> gelu -> W2) + residual -> output.

Projection/MLP matmuls run in float32r (full PE rate at free dim >= 256,
~1.5e-4 rel err). The attention inner path (softmaxed q/k, v, context) runs in
bf16 — full PE rate at any free dim. Per-channel / per-position biases are
preloaded into PSUM with identity/ones matmuls; LN gains and the positional
projections are folded into weights/biases on the host.

Activation-table sets are pinned to natural_log_exp_and_others (stage 1) and
gelu_and_others (stage 2) to avoid ~2.7us table reloads.
"""

import numpy as np

B, N3, N4, DIM, HEADS, MLP_DIM = 256, 256, 64, 512, 8, 2048
N_CORES = 8
BSH = B // N_CORES  # samples per core
EPS = 1e-5
SG4 = 8  # f4-group size (samples per K/V block)
WSCALE = 2.0 ** 10   # fp8 weight scale (weights ~N(0,.02) would underflow fp8)
DESCALE = 1.0 / WSCALE

_BUILD_CACHE = {}


def _host_prep(inputs):
    """Fold LN gains + positional projections into weights/biases (exact)."""
    f = {k: np.asarray(v, dtype=np.float64) for k, v in inputs.items()}
    pos3 = f["pos3"][0]  # [N3, DIM]
    pos4 = f["pos4"][0]  # [N4, DIM]

    import ml_dtypes
    fp8 = ml_dtypes.float8_e4m3

    def q8(w):  # weights are ~N(0, 0.02): scale 2^10 into fp8's sweet spot
        return np.ascontiguousarray(np.clip(w * WSCALE, -240, 240).astype(fp8))

    wq = q8(f["ln1_g"][:, None] * f["Wq"])
    wk = q8(f["ln2_g"][:, None] * f["Wk"])
    wv = q8(f["ln2_g"][:, None] * f["Wv"])
    wr = q8(f["Wr"])
    w1 = q8(f["ln3_g"][:, None] * f["W1"])
    w2 = q8(f["W2"])

    biasq = (((f["ln1_b"][None, :] + pos3) @ f["Wq"] + f["bq"]) * WSCALE).astype(np.float32)
    biask = (((f["ln2_b"][None, :] + pos4) @ f["Wk"] + f["bk"]) * WSCALE).astype(np.float32)
    biasv = ((f["ln2_b"] @ f["Wv"] + f["bv"]) * WSCALE).astype(np.float32)  # [DIM]
    bias1 = (f["ln3_b"] @ f["W1"] + f["b1"]).astype(np.float32)  # [MLP] (post-descale)
    br = (f["br"] * WSCALE).astype(np.float32)
    b2 = (f["b2"] * WSCALE).astype(np.float32)

    # biask in channel-major, tiled over the SG4 samples of an f4-group:
    # [DIM, SG4*N4] with column order (sample_in_group, position)
    biask_cm = np.tile(biask.T[:, None, :], (1, SG4, 1)).reshape(DIM, SG4 * N4)
    biask_cm = np.ascontiguousarray(biask_cm.astype(np.float32))
    # bias1 as [128, 16]: column hc holds biases for hidden channels hc*128..+128
    bias1_cm = np.ascontiguousarray(bias1.reshape(MLP_DIM // 128, 128).T.astype(np.float32))

    return {
        "wq": wq, "wk": wk, "wv": wv, "wr": wr, "w1": w1, "w2": w2,
        "biasq": np.ascontiguousarray(biasq),
        "biask_cm": biask_cm,
        "biasv_row": np.ascontiguousarray(biasv[None, :]),
        "br_row": np.ascontiguousarray(br[None, :]),
        "b2_row": np.ascontiguousarray(b2[None, :]),
        "bias1_cm": bias1_cm,
        "ones_col": np.ones((1, 128), dtype=np.float32),
        "ident": np.eye(128, dtype=np.float32),
        "ident_bf": np.eye(128, dtype=np.float32),  # cast to bf16 tile on chip
    }


def _build(n_samples, repeat=1):
    """Build the Bacc module for one core processing `n_samples` samples.

    `repeat` re-runs the whole computation that many times back-to-back —
    used only for wall-clock timing amplification in test.py."""
    import concourse.bacc as bacc
    import concourse.tile as tile
    import concourse.mybir as mybir
    from concourse.bass import AP  # noqa: F401

    # Restrict activation-table-set choices (see module docstring).
    if not hasattr(bacc, "_orig_get_activation_tables"):
        bacc._orig_get_activation_tables = bacc.get_activation_tables

        def _gat(arch):
            full = bacc._orig_get_activation_tables(arch)
            keep = {"natural_log_exp_and_others", "gelu_and_others"}
            return {n: (s if n in keep else set()) for n, s in full.items()}

        bacc.get_activation_tables = _gat

    F32 = mybir.dt.float32
    F32R = mybir.dt.float32r
    BF16 = mybir.dt.bfloat16
    FP8 = mybir.dt.float8e4
    DR = mybir.MatmulPerfMode.DoubleRow
    AX = mybir.AxisListType.X
    ALU = mybir.AluOpType
    ACTF = mybir.ActivationFunctionType

    NS = n_samples
    assert NS % SG4 == 0
    NG4 = NS // SG4      # f4 groups
    NG2 = NS // 2        # mlp groups of 2 samples

    nc = bacc.Bacc("TRN2", debug=False, num_devices=N_CORES)

    f3 = nc.dram_tensor("f3", [NS, N3, DIM], F32, kind="ExternalInput").ap()
    f4 = nc.dram_tensor("f4", [NS, N4, DIM], F32, kind="ExternalInput").ap()
    wq = nc.dram_tensor("wq", [DIM, DIM], FP8, kind="ExternalInput").ap()
    wk = nc.dram_tensor("wk", [DIM, DIM], FP8, kind="ExternalInput").ap()
    wv = nc.dram_tensor("wv", [DIM, DIM], FP8, kind="ExternalInput").ap()
    wr = nc.dram_tensor("wr", [DIM, DIM], FP8, kind="ExternalInput").ap()
    w1 = nc.dram_tensor("w1", [DIM, MLP_DIM], FP8, kind="ExternalInput").ap()
    w2 = nc.dram_tensor("w2", [MLP_DIM, DIM], FP8, kind="ExternalInput").ap()
    biasq = nc.dram_tensor("biasq", [N3, DIM], F32R, kind="ExternalInput").ap()
    biask_cm = nc.dram_tensor("biask_cm", [DIM, SG4 * N4], F32R, kind="ExternalInput").ap()
    biasv_row = nc.dram_tensor("biasv_row", [1, DIM], F32R, kind="ExternalInput").ap()
    br_row = nc.dram_tensor("br_row", [1, DIM], F32R, kind="ExternalInput").ap()
    b2_row = nc.dram_tensor("b2_row", [1, DIM], F32R, kind="ExternalInput").ap()
    bias1_cm = nc.dram_tensor("bias1_cm", [128, MLP_DIM // 128], F32, kind="ExternalInput").ap()
    ones_col = nc.dram_tensor("ones_col", [1, 128], F32R, kind="ExternalInput").ap()
    ident = nc.dram_tensor("ident", [128, 128], F32R, kind="ExternalInput").ap()
    ident_bf = nc.dram_tensor("ident_bf", [128, 128], F32, kind="ExternalInput").ap()
    out = nc.dram_tensor("out", [NS, N3, DIM], F32, kind="ExternalOutput").ap()

    with tile.TileContext(nc) as tc:
        # ---- pools alive for the whole kernel ----
        with (
            tc.tile_pool(name="consts", bufs=1) as cpool,
            tc.tile_pool(name="wattn", bufs=1) as wpool,
            tc.tile_pool(name="stats", bufs=1) as spool,
            tc.tile_pool(name="dram", bufs=1, space="DRAM") as dpool,
        ):
            ident_sb = cpool.tile([128, 128], F32R, tag="ident")
            nc.sync.dma_start(ident_sb[:], ident)
            identf_sb = cpool.tile([128, 128], F32, tag="identf")
            nc.sync.dma_start(identf_sb[:], ident_bf)
            identb_sb = cpool.tile([128, 128], BF16, tag="identb")
            nc.vector.tensor_copy(identb_sb[:], identf_sb[:])
            eps_sb = cpool.tile([128, 1], F32, tag="eps")
            nc.vector.memset(eps_sb[:], EPS)
            ones_sb = cpool.tile([1, 128], F32R, tag="ones")
            nc.sync.dma_start(ones_sb[:], ones_col)
            bvrow_sb = cpool.tile([1, DIM], F32R, tag="bvrow")
            nc.sync.dma_start(bvrow_sb[:], biasv_row)
            brrow_sb = cpool.tile([1, DIM], F32R, tag="brrow")
            b2row_sb = cpool.tile([1, DIM], F32R, tag="b2row")
            bq_sb = cpool.tile([128, 2, DIM], F32R, tag="bq")
            bk_sb = cpool.tile([128, 4, SG4 * N4], F32R, tag="bk")
            nc.sync.dma_start(bk_sb[:], biask_cm.rearrange("(c p) d -> p c d", p=128))
            b1_sb = cpool.tile([128, MLP_DIM // 128], F32, tag="b1")
            # persistent block-diagonal context tiles (off-diagonal stays zero)
            ctxbd = cpool.tile([128, 4, 128], BF16, tag="ctxbd")
            nc.vector.memset(ctxbd[:], 0.0)

            wq_sb = wpool.tile([128, 4, DIM], FP8, tag="wq")
            wk_sb = wpool.tile([128, 4, DIM], FP8, tag="wk")
            nc.sync.dma_start(wk_sb[:], wk.rearrange("(c p) d -> p c d", p=128))
            wv_sb = wpool.tile([128, 4, DIM], FP8, tag="wv")
            nc.sync.dma_start(wv_sb[:], wv.rearrange("(c p) d -> p c d", p=128))
            wr_sb = wpool.tile([128, 4, DIM], FP8, tag="wr")
            w1_sb = wpool.tile([128, 4, MLP_DIM], FP8, tag="w1")
            w2_sb = wpool.tile([128, 16, DIM], FP8, tag="w2")

            # LN3 (mean, var) per token-chunk column, filled during stage 1
            stats3 = spool.tile([128, 2 * NS, 2], F32, tag="stats3")

            f3o_dram = dpool.tile([NS, N3, DIM], F32, tag="f3spill")

            for _rep in range(repeat):
                # ================= STAGE 1 =================
                with (
                    tc.tile_pool(name="s1_sb", bufs=2) as p1,
                    tc.tile_pool(name="s1_sb3", bufs=3) as p13,
                    tc.tile_pool(name="kv", bufs=3) as pkv,
                    tc.tile_pool(name="ps_tp", bufs=2, space="PSUM") as ps_tp,
                    tc.tile_pool(name="ps_mm", bufs=1, space="PSUM") as ps_mm,
                    tc.tile_pool(name="ps_ctx", bufs=1, space="PSUM") as ps_ctx,
                    tc.tile_pool(name="ps_att", bufs=1, space="PSUM") as ps_att,
                ):
                    def inv_std_from_var(var_view, sinv_view, n_cols, tag):
                        """sinv = exp(-0.5*ln(var+eps)) on [128, n_cols] views."""
                        lnv = p1.tile([128, n_cols], F32, tag=f"lnv_{tag}")
                        nc.scalar.activation(lnv[:], var_view, ACTF.Ln, bias=eps_sb[:])
                        nc.scalar.activation(sinv_view, lnv[:], ACTF.Exp, scale=-0.5)

                    def f4_block(g):
                            # ---------- f4 block: SG4 samples ----------
                            ntb = SG4 // 2  # token-chunks of 128 (2 samples each)
                            x4 = p1.tile([128, ntb, DIM], F32, tag="x4")
                            for t in range(ntb):
                                nc.sync.dma_start(
                                    x4[:, t, :],
                                    f4[SG4 * g + 2 * t: SG4 * g + 2 * t + 2].rearrange(
                                        "a b d -> (a b) d"
                                    ),
                                )
                            mv4 = p1.tile([128, ntb, 2], F32, tag="mv4")
                            for t in range(ntb):
                                bns = p1.tile([128, 6], F32, tag="bns4")
                                nc.vector.bn_stats(bns[:], x4[:, t, :])
                                nc.vector.bn_aggr(mv4[:, t, :], bns[:])
                            negm4 = p1.tile([128, ntb], F32, tag="negm4")
                            nc.vector.tensor_scalar_mul(negm4[:], mv4[:, :, 0], -1.0)
                            sinv4 = p1.tile([128, ntb], F32, tag="sinv4")
                            inv_std_from_var(mv4[:, :, 1], sinv4[:], ntb, "s4")
                            x4h = p1.tile([128, ntb, DIM], BF16, tag="x4h")
                            for t in range(ntb):
                                nc.vector.tensor_scalar(
                                    x4h[:, t, :], x4[:, t, :],
                                    negm4[:, t: t + 1], sinv4[:, t: t + 1],
                                    op0=ALU.add, op1=ALU.mult,
                                )
                            # transpose to channel-major [512ch, SG4*64 tok]
                            x4c = p1.tile([128, 4, SG4 * N4], FP8, tag="x4c", bufs=3)
                            for cc in range(4):
                                ptpf = ps_tp.tile([128, 512], F32R, tag="tp", name="ptpf")
                                ptp = ptpf.bitcast(BF16)
                                for t in range(ntb):
                                    nc.tensor.transpose(
                                        ptp[:, t * 128:(t + 1) * 128],
                                        x4h[:, t, cc * 128:(cc + 1) * 128],
                                        identb_sb[:],
                                    )
                                nc.scalar.activation(x4c[:, cc, :], ptp[:, 0:512], ACTF.Copy)
                            # K projection (channel-major out) + bias preload + exp
                            kx = p1.tile([128, 4, SG4 * N4], BF16, tag="kx")
                            ks = p1.tile([128, 4 * SG4], F32, tag="ks")
                            for cc in range(4):
                                pk = ps_mm.tile([128, 512], F32, tag="mmk", bufs=1)
                                nc.tensor.matmul(
                                    pk[:], ident_sb[:], bk_sb[:, cc, :],
                                    start=True, stop=False,
                                )
                                for kc in range(2):
                                    nc.tensor.matmul(
                                        pk[:],
                                        wk_sb[:, 2 * kc:2 * kc + 2, cc * 128:(cc + 1) * 128],
                                        x4c[:, 2 * kc:2 * kc + 2, :],
                                        start=False, stop=(kc == 1),
                                        perf_mode=DR,
                                    )
                                nc.scalar.activation(kx[:, cc, :], pk[:], ACTF.Exp, scale=DESCALE)
                                nc.vector.reduce_sum(
                                    ks[:, cc * SG4:(cc + 1) * SG4],
                                    kx[:, cc, :].rearrange("p (s d) -> p s d", s=SG4),
                                    axis=AX,
                                )
                            kr = pkv.tile([128, 4 * SG4], F32, tag="kr")
                            nc.vector.reciprocal(kr[:], ks[:])
                            # V projection (token-major out) + bias preload
                            v_tm = pkv.tile([128, ntb, DIM], BF16, tag="v_tm")
                            for t in range(ntb):
                                pv = ps_mm.tile([128, 512], F32, tag="mmk", bufs=1)
                                nc.tensor.matmul(pv[:], ones_sb[:], bvrow_sb[:], start=True, stop=False)
                                for kc in range(2):
                                    nc.tensor.matmul(
                                        pv[:],
                                        x4c[:, 2 * kc:2 * kc + 2, t * 128:(t + 1) * 128],
                                        wv_sb[:, 2 * kc:2 * kc + 2, :],
                                        start=False, stop=(kc == 1),
                                        perf_mode=DR,
                                    )
                                nc.scalar.activation(v_tm[:, t, :], pv[:], ACTF.Copy, scale=DESCALE)
                            # k back to token-major (bf16)
                            k_tm = pkv.tile([128, ntb, DIM], BF16, tag="k_tm")
                            for t in range(ntb):
                                ptp2f = ps_tp.tile([128, 512], F32R, tag="tp", name="ptp2f")
                                ptp2 = ptp2f.bitcast(BF16)
                                for cc in range(4):
                                    nc.tensor.transpose(
                                        ptp2[:, cc * 128:(cc + 1) * 128],
                                        kx[:, cc, t * 128:(t + 1) * 128],
                                        identb_sb[:],
                                    )
                                nc.vector.tensor_copy(k_tm[:, t, :], ptp2[:, 0:512])

                            return k_tm, v_tm, kr

                    def x3_load(s):
                        x3 = p13.tile([128, 2, DIM], F32, tag="x3", name="x3")
                        for t in range(2):
                            nc.sync.dma_start(
                                x3[:, t, :], f3[s, t * 128:(t + 1) * 128, :]
                            )
                        return x3

                    x3state = {}
                    x3state[0] = x3_load(0)
                    x3state[1] = x3_load(1)
                    kvstate = {}
                    kvstate[0] = f4_block(0)
                    # deferred loads: first needed ~8-12us in (Q/Wr of sample 0)
                    nc.sync.dma_start(bq_sb[:], biasq.rearrange("(t p) d -> p t d", p=128))
                    nc.sync.dma_start(wq_sb[:], wq.rearrange("(c p) d -> p c d", p=128))
                    nc.sync.dma_start(wr_sb[:], wr.rearrange("(c p) d -> p c d", p=128))
                    nc.sync.dma_start(brrow_sb[:], br_row)
                    if NG4 > 1:
                        kvstate[1] = f4_block(1)
                    # W1/W2 loads deferred here so startup DMA bandwidth goes
                    # to the first groups' activations and attention weights.
                    nc.sync.dma_start(w1_sb[:], w1.rearrange("(c p) d -> p c d", p=128))
                    nc.sync.dma_start(w2_sb[:], w2.rearrange("(c p) d -> p c d", p=128))
                    nc.sync.dma_start(b1_sb[:], bias1_cm)
                    nc.sync.dma_start(b2row_sb[:], b2_row)
                    for g in range(NG4):
                        k_tm, v_tm, kr = kvstate.pop(g)
                        # ---------- f3 blocks: SG4 samples ----------
                        for si in range(SG4):
                            if si == 4 and g + 2 < NG4:
                                kvstate[g + 2] = f4_block(g + 2)
                            s = SG4 * g + si
                            tb = si // 2
                            pb = (si % 2) * 64
                            x3 = x3state.pop(s)
                            if s + 2 < NS:
                                x3state[s + 2] = x3_load(s + 2)
                            mv1 = p1.tile([128, 2, 2], F32, tag="mv1")
                            for t in range(2):
                                bns1 = p1.tile([128, 6], F32, tag="bns1")
                                nc.vector.bn_stats(bns1[:], x3[:, t, :])
                                nc.vector.bn_aggr(mv1[:, t, :], bns1[:])
                            negm1 = p1.tile([128, 2], F32, tag="negm1")
                            nc.vector.tensor_scalar_mul(negm1[:], mv1[:, :, 0], -1.0)
                            sinv1 = p1.tile([128, 2], F32, tag="sinv1")
                            inv_std_from_var(mv1[:, :, 1], sinv1[:], 2, "s1")
                            x3h = p1.tile([128, 2, DIM], BF16, tag="x3h", bufs=3)
                            for t in range(2):
                                nc.vector.tensor_scalar(
                                    x3h[:, t, :], x3[:, t, :],
                                    negm1[:, t: t + 1], sinv1[:, t: t + 1],
                                    op0=ALU.add, op1=ALU.mult,
                                )
                            x3c = p1.tile([128, 4, 256], FP8, tag="x3c", bufs=4)
                            for cc in range(4):
                                ptpf3 = ps_tp.tile([128, 512], F32R, tag="tp", name="ptpf3")
                                ptp = ptpf3.bitcast(BF16)
                                for t in range(2):
                                    nc.tensor.transpose(
                                        ptp[:, t * 128:(t + 1) * 128],
                                        x3h[:, t, cc * 128:(cc + 1) * 128],
                                        identb_sb[:],
                                    )
                                nc.vector.tensor_copy(x3c[:, cc, :], ptp[:, 0:256])
                            # Q projection + biasq preload, then exp over both chunks
                            e_tm = p1.tile([128, 2, DIM], BF16, tag="e_tm", bufs=3)
                            for t in range(2):
                                pq = ps_mm.tile([128, 512], F32, tag="mmq", name="pq", bufs=3)
                                nc.tensor.matmul(
                                    pq[:], ident_sb[:], bq_sb[:, t, :],
                                    start=True, stop=False,
                                )
                                for kc in range(2):
                                    nc.tensor.matmul(
                                        pq[:],
                                        x3c[:, 2 * kc:2 * kc + 2, t * 128:(t + 1) * 128],
                                        wq_sb[:, 2 * kc:2 * kc + 2, :],
                                        start=False, stop=(kc == 1),
                                        perf_mode=DR,
                                    )
                                nc.scalar.activation(e_tm[:, t, :], pq[:], ACTF.Exp, scale=DESCALE)
                            qs = p1.tile([128, 16], F32, tag="qs")
                            nc.vector.reduce_sum(
                                qs[:],
                                e_tm.rearrange("p a (h d) -> p (a h) d", h=8)[:],
                                axis=AX,
                            )
                            qr = p1.tile([128, 16], F32, tag="qr")
                            nc.vector.reciprocal(qr[:], qs[:])
                            q_tm = p1.tile([128, 2, DIM], BF16, tag="q_tm", bufs=3)
                            nc.vector.tensor_tensor(
                                q_tm.rearrange("p a (h d) -> p (a h) d", h=8)[:],
                                e_tm.rearrange("p a (h d) -> p (a h) d", h=8)[:],
                                qr[:].unsqueeze(-1).broadcast_to([128, 16, 64]),
                                op=ALU.mult,
                            )
                            q_cm = p1.tile([128, 4, 256], BF16, tag="q_cm", bufs=4)
                            for cc in range(4):
                                ptpbf = ps_tp.tile([128, 512], F32R, tag="tp", name="ptpbf")
                                ptpb = ptpbf.bitcast(BF16)
                                for t in range(2):
                                    nc.tensor.transpose(
                                        ptpb[:, t * 128:(t + 1) * 128],
                                        q_tm[:, t, cc * 128:(cc + 1) * 128],
                                        identb_sb[:],
                                    )
                                nc.vector.tensor_copy(q_cm[:, cc, :], ptpb[:, 0:256])
                            # attention per head-pair
                            att_cm = p1.tile([128, 4, 256], FP8, tag="att_cm")
                            for hp in range(4):
                                pctx = ps_ctx.tile([128, 128], F32, tag="ctx")
                                nc.tensor.matmul(
                                    pctx[:],
                                    k_tm[pb:pb + 64, tb, hp * 128:(hp + 1) * 128],
                                    v_tm[pb:pb + 64, tb, hp * 128:(hp + 1) * 128],
                                    start=True, stop=True,
                                )
                                for hh in range(2):
                                    nc.vector.tensor_scalar_mul(
                                        ctxbd[hh * 64:(hh + 1) * 64, hp, hh * 64:(hh + 1) * 64],
                                        pctx[hh * 64:(hh + 1) * 64, hh * 64:(hh + 1) * 64],
                                        kr[hh * 64:(hh + 1) * 64, hp * SG4 + si: hp * SG4 + si + 1],
                                    )
                                patt = ps_att.tile([128, 256], F32, tag="att")
                                nc.tensor.matmul(
                                    patt[:], ctxbd[:, hp, :], q_cm[:, hp, :],
                                    start=True, stop=True,
                                )
                                nc.scalar.activation(att_cm[:, hp, :], patt[:], ACTF.Copy)
                            # Wr + residual -> f3out (+ LN3 stats via bn_stats)
                            for t in range(2):
                                po = ps_mm.tile([128, 512], F32, tag="mmq", bufs=3)
                                nc.tensor.matmul(po[:], ones_sb[:], brrow_sb[:], start=True, stop=False)
                                for cc in range(2):
                                    nc.tensor.matmul(
                                        po[:],
                                        att_cm[:, 2 * cc:2 * cc + 2, t * 128:(t + 1) * 128],
                                        wr_sb[:, 2 * cc:2 * cc + 2, :],
                                        start=False, stop=(cc == 1),
                                        perf_mode=DR,
                                    )
                                f3o = p13.tile([128, DIM], F32, tag="f3o")
                                nc.vector.scalar_tensor_tensor(
                                    f3o[:], po[:], DESCALE, x3[:, t, :],
                                    op0=ALU.mult, op1=ALU.add,
                                )
                                bns3 = p1.tile([128, 6], F32, tag="bns3")
                                nc.vector.bn_stats(bns3[:], f3o[:])
                                nc.vector.bn_aggr(stats3[:, 2 * s + t, :], bns3[:])
                                nc.sync.dma_start(
                                    f3o_dram[s, t * 128:(t + 1) * 128, :], f3o[:]
                                )

                # ================= STAGE 2 =================
                with (
                    tc.tile_pool(name="s2_sb", bufs=3) as p2,
                    tc.tile_pool(name="s2_sb3", bufs=3) as p23,
                    tc.tile_pool(name="ps2_tp", bufs=2, space="PSUM") as ps2_tp,
                    tc.tile_pool(name="ps2_w1", bufs=2, space="PSUM") as ps2_w1,
                    tc.tile_pool(name="ps2_w2", bufs=4, space="PSUM") as ps2_w2,
                ):
                    # LN3 stats math for all samples at once
                    negm3 = p2.tile([128, 2 * NS], F32, tag="negm3")
                    nc.vector.tensor_scalar_mul(negm3[:], stats3[:, :, 0], -1.0)
                    lnv3 = p2.tile([128, 2 * NS], F32, tag="lnv3")
                    nc.scalar.activation(lnv3[:], stats3[:, :, 1], ACTF.Ln, bias=eps_sb[:])
                    s3 = p2.tile([128, 2 * NS], F32, tag="s3")
                    nc.scalar.activation(s3[:], lnv3[:], ACTF.Exp, scale=-0.5)

                    for g in range(NG2):
                        f3o2 = p23.tile([128, 4, DIM], F32, tag="f3o2")
                        for c in range(4):
                            nc.sync.dma_start(
                                f3o2[:, c, :],
                                f3o_dram[2 * g + c // 2, (c % 2) * 128:(c % 2) * 128 + 128, :],
                            )
                        xoh = p2.tile([128, 4, DIM], BF16, tag="xoh")
                        for c in range(4):
                            col = 4 * g + c
                            nc.vector.tensor_scalar(
                                xoh[:, c, :], f3o2[:, c, :],
                                negm3[:, col: col + 1], s3[:, col: col + 1],
                                op0=ALU.add, op1=ALU.mult,
                            )
                        xoc = p2.tile([128, 4, DIM], FP8, tag="xoc")
                        for cc in range(4):
                            ptpf2 = ps2_tp.tile([128, 512], F32R, tag="tp2", name="ptpf2")
                            ptp = ptpf2.bitcast(BF16)
                            for c in range(4):
                                nc.tensor.transpose(
                                    ptp[:, c * 128:(c + 1) * 128],
                                    xoh[:, c, cc * 128:(cc + 1) * 128],
                                    identb_sb[:],
                                )
                            nc.scalar.activation(xoc[:, cc, :], ptp[:, 0:512], ACTF.Copy)
                        pf = []
                        for c in range(4):
                            pfc = ps2_w2.tile([128, 512], F32, tag="w2acc")
                            nc.tensor.matmul(pfc[:], ones_sb[:], b2row_sb[:], start=True, stop=False)
                            pf.append(pfc)
                        for hp in range(8):
                            gt2 = p23.tile([128, 2, DIM], FP8, tag="gt")
                            for i in range(2):
                                hc = 2 * hp + i
                                pw1 = ps2_w1.tile([128, 512], F32, tag="w1ps")
                                for kc in range(2):
                                    nc.tensor.matmul(
                                        pw1[:],
                                        w1_sb[:, 2 * kc:2 * kc + 2, hc * 128:(hc + 1) * 128],
                                        xoc[:, 2 * kc:2 * kc + 2, :],
                                        start=(kc == 0), stop=(kc == 1),
                                        perf_mode=DR,
                                    )
                                nc.scalar.activation(
                                    gt2[:, i, :], pw1[:], ACTF.Gelu,
                                    bias=b1_sb[:, hc: hc + 1], scale=DESCALE,
                                )
                            for c in range(4):
                                nc.tensor.matmul(
                                    pf[c][:],
                                    gt2[:, :, c * 128:(c + 1) * 128],
                                    w2_sb[:, 2 * hp:2 * hp + 2, :],
                                    start=False, stop=(hp == 7),
                                    perf_mode=DR,
                                    skip_group_check=True,
                                )
                        for c in range(4):
                            outt = p2.tile([128, DIM], F32, tag="outt")
                            nc.vector.scalar_tensor_tensor(
                                outt[:], pf[c][:], DESCALE, f3o2[:, c, :],
                                op0=ALU.mult, op1=ALU.add,
                            )
                            nc.sync.dma_start(
                                out[2 * g + c // 2, (c % 2) * 128:(c % 2) * 128 + 128, :],
                                outt[:],
                            )

    nc.compile()
    return nc


def _get_module(n_samples):
    if n_samples not in _BUILD_CACHE:
        _BUILD_CACHE[n_samples] = _build(n_samples)
    return _BUILD_CACHE[n_samples]


def kernel(**inputs) -> np.ndarray:
    from concourse.bass_utils import run_bass_kernel_spmd

    consts = _host_prep(inputs)
    f3 = np.ascontiguousarray(np.asarray(inputs["f3"], dtype=np.float32))
    f4 = np.ascontiguousarray(np.asarray(inputs["f4"], dtype=np.float32))

    nc = _get_module(BSH)
    in_maps = []
    for c in range(N_CORES):
        m = dict(consts)
        m["f3"] = np.ascontiguousarray(f3[c * BSH:(c + 1) * BSH])
        m["f4"] = np.ascontiguousarray(f4[c * BSH:(c + 1) * BSH])
        in_maps.append(m)
    res = run_bass_kernel_spmd(nc, in_maps, core_ids=list(range(N_CORES)))
    return np.concatenate([res.results[c]["out"] for c in range(N_CORES)], axis=0)



# revision 21
# speedup vs baseline: 3.1187x; 1.0702x over previous
"""Trainium2 Bass kernel for nn_CPA_43 (dense transformer block, CPA attention).

Data-parallel over batch: B=256 sharded as 32 samples per core across 8 cores.
All weights replicated. Two on-chip stages per core:
  stage 1: LN1/LN2, Q/K/V projections, channel-softmax(q), position-softmax(k),
           context/attention matmuls, Wr + residual -> f3out (spilled to DRAM),
           LN3 statistics on the fly (bn_stats).
  stage 2: LN3 apply, MLP (W1 -> gelu -> W2) + residual -> output.

Projection/MLP matmuls run in float32r (full PE rate at free dim >= 256,
~1.5e-4 rel err). The attention inner path (softmaxed q/k, v, context) runs in
bf16 — full PE rate at any free dim. Per-channel / per-position biases are
preloaded into PSUM with identity/ones matmuls; LN gains and the positional
projections are folded into weights/biases on the host.

Activation-table sets are pinned to natural_log_exp_and_others (stage 1) and
gelu_and_others (stage 2) to avoid ~2.7us table reloads.
"""

import numpy as np

B, N3, N4, DIM, HEADS, MLP_DIM = 256, 256, 64, 512, 8, 2048
N_CORES = 8
BSH = B // N_CORES  # samples per core
EPS = 1e-5
SG4 = 8  # f4-group size (samples per K/V block)
WSCALE = 2.0 ** 10   # fp8 weight scale (weights ~N(0,.02) would underflow fp8)
DESCALE = 1.0 / WSCALE

_BUILD_CACHE = {}


def _host_prep(inputs):
    """Fold LN gains + positional projections into weights/biases (exact)."""
    f = {k: np.asarray(v, dtype=np.float64) for k, v in inputs.items()}
    pos3 = f["pos3"][0]  # [N3, DIM]
    pos4 = f["pos4"][0]  # [N4, DIM]

    import ml_dtypes
    fp8 = ml_dtypes.float8_e4m3

    def q8(w):  # weights are ~N(0, 0.02): scale 2^10 into fp8's sweet spot
        return np.ascontiguousarray(np.clip(w * WSCALE, -240, 240).astype(fp8))

    bf16 = ml_dtypes.bfloat16
    wq = np.ascontiguousarray((f["ln1_g"][:, None] * f["Wq"]).astype(bf16))
    wk = np.ascontiguousarray((f["ln2_g"][:, None] * f["Wk"]).astype(bf16))
    wv = np.ascontiguousarray((f["ln2_g"][:, None] * f["Wv"]).astype(bf16))
    wr = q8(f["Wr"])
    w1 = q8(f["ln3_g"][:, None] * f["W1"])
    w2 = q8(f["W2"])

    biasq = ((f["ln1_b"][None, :] + pos3) @ f["Wq"] + f["bq"]).astype(np.float32)
    biask = ((f["ln2_b"][None, :] + pos4) @ f["Wk"] + f["bk"]).astype(np.float32)
    bias1 = (f["ln3_b"] @ f["W1"] + f["b1"]).astype(np.float32)  # [MLP] (post-descale)
    # The V bias passes through attention unchanged (softmax(k) sums to 1 over
    # positions, softmax(q) sums to 1 over head channels), so fold it into br.
    biasv = f["ln2_b"] @ f["Wv"] + f["bv"]  # [DIM]
    br = ((f["br"] + biasv @ f["Wr"]) * WSCALE).astype(np.float32)
    b2 = (f["b2"] * WSCALE).astype(np.float32)

    # biask in channel-major, tiled over the SG4 samples of an f4-group:
    # [DIM, SG4*N4] with column order (sample_in_group, position)
    biask_cm = np.tile(biask.T[:, None, :], (1, SG4, 1)).reshape(DIM, SG4 * N4)
    biask_cm = np.ascontiguousarray(biask_cm.astype(np.float32))
    # bias1 as [128, 16]: column hc holds biases for hidden channels hc*128..+128
    bias1_cm = np.ascontiguousarray(bias1.reshape(MLP_DIM // 128, 128).T.astype(np.float32))

    return {
        "wq": wq, "wk": wk, "wv": wv, "wr": wr, "w1": w1, "w2": w2,
        "biasq": np.ascontiguousarray(biasq),
        "biask_cm": biask_cm,
        "br_row": np.ascontiguousarray(br[None, :]),
        "b2_row": np.ascontiguousarray(b2[None, :]),
        "bias1_cm": bias1_cm,
        "ones_col": np.ones((1, 128), dtype=np.float32),
        "ident": np.eye(128, dtype=np.float32),
        "ident_bf": np.eye(128, dtype=np.float32),  # cast to bf16 tile on chip
    }


def _build(n_samples, repeat=1):
    """Build the Bacc module for one core processing `n_samples` samples.

    `repeat` re-runs the whole computation that many times back-to-back —
    used only for wall-clock timing amplification in test.py."""
    import concourse.bacc as bacc
    import concourse.tile as tile
    import concourse.mybir as mybir
    from concourse.bass import AP  # noqa: F401

    # Restrict activation-table-set choices (see module docstring).
    if not hasattr(bacc, "_orig_get_activation_tables"):
        bacc._orig_get_activation_tables = bacc.get_activation_tables

        def _gat(arch):
            full = bacc._orig_get_activation_tables(arch)
            keep = {"natural_log_exp_and_others", "gelu_and_others"}
            return {n: (s if n in keep else set()) for n, s in full.items()}

        bacc.get_activation_tables = _gat

    F32 = mybir.dt.float32
    F32R = mybir.dt.float32r
    BF16 = mybir.dt.bfloat16
    FP8 = mybir.dt.float8e4
    DR = mybir.MatmulPerfMode.DoubleRow
    AX = mybir.AxisListType.X
    ALU = mybir.AluOpType
    ACTF = mybir.ActivationFunctionType

    NS = n_samples
    assert NS % SG4 == 0
    NG4 = NS // SG4      # f4 groups
    NG2 = NS // 2        # mlp groups of 2 samples

    nc = bacc.Bacc("TRN2", debug=False, num_devices=N_CORES)

    f3 = nc.dram_tensor("f3", [NS, N3, DIM], F32, kind="ExternalInput").ap()
    f4 = nc.dram_tensor("f4", [NS, N4, DIM], F32, kind="ExternalInput").ap()
    wq = nc.dram_tensor("wq", [DIM, DIM], BF16, kind="ExternalInput").ap()
    wk = nc.dram_tensor("wk", [DIM, DIM], BF16, kind="ExternalInput").ap()
    wv = nc.dram_tensor("wv", [DIM, DIM], BF16, kind="ExternalInput").ap()
    wr = nc.dram_tensor("wr", [DIM, DIM], FP8, kind="ExternalInput").ap()
    w1 = nc.dram_tensor("w1", [DIM, MLP_DIM], FP8, kind="ExternalInput").ap()
    w2 = nc.dram_tensor("w2", [MLP_DIM, DIM], FP8, kind="ExternalInput").ap()
    biasq = nc.dram_tensor("biasq", [N3, DIM], F32R, kind="ExternalInput").ap()
    biask_cm = nc.dram_tensor("biask_cm", [DIM, SG4 * N4], F32R, kind="ExternalInput").ap()
    br_row = nc.dram_tensor("br_row", [1, DIM], F32R, kind="ExternalInput").ap()
    b2_row = nc.dram_tensor("b2_row", [1, DIM], F32R, kind="ExternalInput").ap()
    bias1_cm = nc.dram_tensor("bias1_cm", [128, MLP_DIM // 128], F32, kind="ExternalInput").ap()
    ones_col = nc.dram_tensor("ones_col", [1, 128], F32R, kind="ExternalInput").ap()
    ident = nc.dram_tensor("ident", [128, 128], F32R, kind="ExternalInput").ap()
    out = nc.dram_tensor("out", [NS, N3, DIM], F32, kind="ExternalOutput").ap()

    with tile.TileContext(nc) as tc:
        # ---- pools alive for the whole kernel ----
        with (
            tc.tile_pool(name="consts", bufs=1) as cpool,
            tc.tile_pool(name="wattn", bufs=1) as wpool,
            tc.tile_pool(name="stats", bufs=1) as spool,
            tc.tile_pool(name="dram", bufs=1, space="DRAM") as dpool,
        ):
            ident_sb = cpool.tile([128, 128], F32R, tag="ident")
            nc.sync.dma_start(ident_sb[:], ident)
            eps_sb = cpool.tile([128, 1], F32, tag="eps")
            nc.vector.memset(eps_sb[:], EPS)
            ones_sb = cpool.tile([1, 128], F32R, tag="ones")
            nc.sync.dma_start(ones_sb[:], ones_col)
            brrow_sb = cpool.tile([1, DIM], F32R, tag="brrow")
            b2row_sb = cpool.tile([1, DIM], F32R, tag="b2row")
            bq_sb = cpool.tile([128, 2, DIM], F32R, tag="bq")
            bk_sb = cpool.tile([128, 4, SG4 * N4], F32R, tag="bk")
            nc.sync.dma_start(bk_sb[:], biask_cm.rearrange("(c p) d -> p c d", p=128))
            b1_sb = cpool.tile([128, MLP_DIM // 128], F32, tag="b1")
            # persistent block-diagonal context tiles (off-diagonal stays zero)
            ctxbd = cpool.tile([128, 4, 128], BF16, tag="ctxbd")
            nc.vector.memset(ctxbd[:], 0.0)

            wq_sb = wpool.tile([128, 4, DIM], BF16, tag="wq")
            wk_sb = wpool.tile([128, 4, DIM], BF16, tag="wk")
            nc.sync.dma_start(wk_sb[:], wk.rearrange("(c p) d -> p c d", p=128))
            wv_sb = wpool.tile([128, 4, DIM], BF16, tag="wv")
            nc.sync.dma_start(wv_sb[:], wv.rearrange("(c p) d -> p c d", p=128))
            wr_sb = wpool.tile([128, 4, DIM], FP8, tag="wr")
            w1_sb = wpool.tile([128, 4, MLP_DIM], FP8, tag="w1")
            w2_sb = wpool.tile([128, 16, DIM], FP8, tag="w2")

            # LN3 (mean, var) per token-chunk column, filled during stage 1
            stats3 = spool.tile([128, 2 * NS, 2], F32, tag="stats3")

            f3o_dram = dpool.tile([NS, N3, DIM], BF16, tag="f3spill")

            for _rep in range(repeat):
                # ================= STAGE 1 =================
                with (
                    tc.tile_pool(name="s1_sb", bufs=2) as p1,
                    tc.tile_pool(name="s1_sb3", bufs=3) as p13,
                    tc.tile_pool(name="kv", bufs=3) as pkv,
                    tc.tile_pool(name="ps_mm", bufs=1, space="PSUM") as ps_mm,
                    tc.tile_pool(name="ps_ctx", bufs=1, space="PSUM") as ps_ctx,
                    tc.tile_pool(name="ps_att", bufs=1, space="PSUM") as ps_att,
                ):
                    def inv_std_from_var(var_view, sinv_view, n_cols, tag):
                        """sinv = exp(-0.5*ln(var+eps)) on [128, n_cols] views."""
                        lnv = p1.tile([128, n_cols], F32, tag=f"lnv_{tag}")
                        nc.scalar.activation(lnv[:], var_view, ACTF.Ln, bias=eps_sb[:])
                        nc.scalar.activation(sinv_view, lnv[:], ACTF.Exp, scale=-0.5)

                    def f4_block(g):
                            # ---------- f4 block: SG4 samples ----------
                            ntb = SG4 // 2  # token-chunks of 128 (2 samples each)
                            x4 = p1.tile([128, ntb, DIM], F32, tag="x4")
                            for t in range(ntb):
                                nc.sync.dma_start(
                                    x4[:, t, :],
                                    f4[SG4 * g + 2 * t: SG4 * g + 2 * t + 2].rearrange(
                                        "a b d -> (a b) d"
                                    ),
                                )
                            mv4 = p1.tile([128, ntb, 2], F32, tag="mv4")
                            for t in range(ntb):
                                bns = p1.tile([128, 6], F32, tag="bns4")
                                nc.vector.bn_stats(bns[:], x4[:, t, :])
                                nc.vector.bn_aggr(mv4[:, t, :], bns[:])
                            negm4 = p1.tile([128, ntb], F32, tag="negm4")
                            nc.vector.tensor_scalar_mul(negm4[:], mv4[:, :, 0], -1.0)
                            sinv4 = p1.tile([128, ntb], F32, tag="sinv4")
                            inv_std_from_var(mv4[:, :, 1], sinv4[:], ntb, "s4")
                            x4h = p1.tile([128, ntb, DIM], BF16, tag="x4h")
                            for t in range(ntb):
                                nc.vector.tensor_scalar(
                                    x4h[:, t, :], x4[:, t, :],
                                    negm4[:, t: t + 1], sinv4[:, t: t + 1],
                                    op0=ALU.add, op1=ALU.mult,
                                )
                            # channel-major [512ch, SG4*64 tok] via DMA XBAR
                            x4c = p1.tile([128, 4, SG4 * N4], BF16, tag="x4c", bufs=3)
                            for t in range(ntb):
                                nc.sync.dma_start_transpose(
                                    x4c[:, :, t * 128:(t + 1) * 128], x4h[:, t, :]
                                )
                            # K projection (channel-major out) + bias preload + exp
                            kx = p1.tile([128, 4, SG4 * N4], BF16, tag="kx")
                            ks = p1.tile([128, 4 * SG4], F32, tag="ks")
                            for cc in range(4):
                                pk = ps_mm.tile([128, 512], F32, tag="mmk", bufs=1)
                                nc.tensor.matmul(
                                    pk[:], ident_sb[:], bk_sb[:, cc, :],
                                    start=True, stop=False,
                                )
                                for kc in range(4):
                                    nc.tensor.matmul(
                                        pk[:],
                                        wk_sb[:, kc, cc * 128:(cc + 1) * 128],
                                        x4c[:, kc, :],
                                        start=False, stop=(kc == 3),
                                    )
                                nc.scalar.activation(kx[:, cc, :], pk[:], ACTF.Exp)
                                nc.vector.reduce_sum(
                                    ks[:, cc * SG4:(cc + 1) * SG4],
                                    kx[:, cc, :].rearrange("p (s d) -> p s d", s=SG4),
                                    axis=AX,
                                )
                            kr = pkv.tile([128, 4 * SG4], F32, tag="kr")
                            nc.vector.reciprocal(kr[:], ks[:])
                            # V projection (token-major out) + bias preload
                            v_tm = pkv.tile([128, ntb, DIM], BF16, tag="v_tm")
                            for t in range(ntb):
                                pv = ps_mm.tile([128, 512], F32, tag="mmk", bufs=1)
                                for kc in range(4):
                                    nc.tensor.matmul(
                                        pv[:],
                                        x4c[:, kc, t * 128:(t + 1) * 128],
                                        wv_sb[:, kc, :],
                                        start=(kc == 0), stop=(kc == 3),
                                    )
                                nc.scalar.activation(v_tm[:, t, :], pv[:], ACTF.Copy)
                            # k back to token-major (bf16) via DMA XBAR transpose
                            k_tm = pkv.tile([128, ntb, DIM], BF16, tag="k_tm")
                            for cc in range(4):
                                nc.sync.dma_start_transpose(
                                    k_tm[:, :, cc * 128:(cc + 1) * 128], kx[:, cc, :]
                                )

                            return k_tm, v_tm, kr

                    def x3_load(s):
                        x3 = p13.tile([128, 2, DIM], F32, tag="x3", name="x3")
                        for t in range(2):
                            nc.sync.dma_start(
                                x3[:, t, :], f3[s, t * 128:(t + 1) * 128, :]
                            )
                        return x3

                    x3state = {}
                    x3state[0] = x3_load(0)
                    x3state[1] = x3_load(1)
                    kvstate = {}
                    kvstate[0] = f4_block(0)
                    # deferred loads: first needed ~8-12us in (Q/Wr of sample 0)
                    nc.sync.dma_start(bq_sb[:], biasq.rearrange("(t p) d -> p t d", p=128))
                    nc.sync.dma_start(wq_sb[:], wq.rearrange("(c p) d -> p c d", p=128))
                    nc.sync.dma_start(wr_sb[:], wr.rearrange("(c p) d -> p c d", p=128))
                    nc.sync.dma_start(brrow_sb[:], br_row)
                    if NG4 > 1:
                        kvstate[1] = f4_block(1)
                    # W1/W2 loads deferred here so startup DMA bandwidth goes
                    # to the first groups' activations and attention weights.
                    nc.sync.dma_start(w1_sb[:], w1.rearrange("(c p) d -> p c d", p=128))
                    nc.sync.dma_start(w2_sb[:], w2.rearrange("(c p) d -> p c d", p=128))
                    nc.sync.dma_start(b1_sb[:], bias1_cm)
                    nc.sync.dma_start(b2row_sb[:], b2_row)
                    for g in range(NG4):
                        k_tm, v_tm, kr = kvstate.pop(g)
                        # ---------- f3 blocks: SG4 samples ----------
                        for si in range(SG4):
                            if si == 4 and g + 2 < NG4:
                                kvstate[g + 2] = f4_block(g + 2)
                            s = SG4 * g + si
                            tb = si // 2
                            pb = (si % 2) * 64
                            x3 = x3state.pop(s)
                            if s + 2 < NS:
                                x3state[s + 2] = x3_load(s + 2)
                            mv1 = p1.tile([128, 2, 2], F32, tag="mv1")
                            for t in range(2):
                                bns1 = p1.tile([128, 6], F32, tag="bns1")
                                nc.vector.bn_stats(bns1[:], x3[:, t, :])
                                nc.vector.bn_aggr(mv1[:, t, :], bns1[:])
                            negm1 = p1.tile([128, 2], F32, tag="negm1")
                            nc.vector.tensor_scalar_mul(negm1[:], mv1[:, :, 0], -1.0)
                            sinv1 = p1.tile([128, 2], F32, tag="sinv1")
                            inv_std_from_var(mv1[:, :, 1], sinv1[:], 2, "s1")
                            x3h = p1.tile([128, 2, DIM], BF16, tag="x3h", bufs=3)
                            for t in range(2):
                                nc.vector.tensor_scalar(
                                    x3h[:, t, :], x3[:, t, :],
                                    negm1[:, t: t + 1], sinv1[:, t: t + 1],
                                    op0=ALU.add, op1=ALU.mult,
                                )
                            x3c = p1.tile([128, 4, 256], BF16, tag="x3c", bufs=4)
                            for t in range(2):
                                nc.sync.dma_start_transpose(
                                    x3c[:, :, t * 128:(t + 1) * 128], x3h[:, t, :]
                                )
                            # Q projection + biasq preload, then exp over both chunks
                            e_tm = p1.tile([128, 2, DIM], BF16, tag="e_tm", bufs=3)
                            for t in range(2):
                                pq = ps_mm.tile([128, 512], F32, tag="mmq", name="pq", bufs=3)
                                nc.tensor.matmul(
                                    pq[:], ident_sb[:], bq_sb[:, t, :],
                                    start=True, stop=False,
                                )
                                for kc in range(4):
                                    nc.tensor.matmul(
                                        pq[:],
                                        x3c[:, kc, t * 128:(t + 1) * 128],
                                        wq_sb[:, kc, :],
                                        start=False, stop=(kc == 3),
                                    )
                                nc.scalar.activation(e_tm[:, t, :], pq[:], ACTF.Exp)
                            qs = p1.tile([128, 16], F32, tag="qs")
                            nc.vector.reduce_sum(
                                qs[:],
                                e_tm.rearrange("p a (h d) -> p (a h) d", h=8)[:],
                                axis=AX,
                            )
                            qr = p1.tile([128, 16], BF16, tag="qr")
                            with nc.allow_low_precision(reason="softmax norm 1/sum in bf16"):
                                nc.vector.reciprocal(qr[:], qs[:])
                            q_tm = p1.tile([128, 2, DIM], BF16, tag="q_tm", bufs=3)
                            nc.vector.tensor_tensor(
                                q_tm.rearrange("p a (h d) -> p (a h) d", h=8)[:],
                                e_tm.rearrange("p a (h d) -> p (a h) d", h=8)[:],
                                qr[:].unsqueeze(-1).broadcast_to([128, 16, 64]),
                                op=ALU.mult,
                            )
                            q_cm = p1.tile([128, 4, 256], BF16, tag="q_cm", bufs=4)
                            for t in range(2):
                                nc.sync.dma_start_transpose(
                                    q_cm[:, :, t * 128:(t + 1) * 128], q_tm[:, t, :]
                                )
                            # attention per head-pair
                            att_cm = p1.tile([128, 4, 256], FP8, tag="att_cm")
                            for hp in range(4):
                                pctx = ps_ctx.tile([128, 128], F32, tag="ctx")
                                nc.tensor.matmul(
                                    pctx[:],
                                    k_tm[pb:pb + 64, tb, hp * 128:(hp + 1) * 128],
                                    v_tm[pb:pb + 64, tb, hp * 128:(hp + 1) * 128],
                                    start=True, stop=True,
                                )
                                for hh in range(2):
                                    nc.vector.tensor_scalar_mul(
                                        ctxbd[hh * 64:(hh + 1) * 64, hp, hh * 64:(hh + 1) * 64],
                                        pctx[hh * 64:(hh + 1) * 64, hh * 64:(hh + 1) * 64],
                                        kr[hh * 64:(hh + 1) * 64, hp * SG4 + si: hp * SG4 + si + 1],
                                    )
                                patt = ps_att.tile([128, 256], F32, tag="att")
                                nc.tensor.matmul(
                                    patt[:], ctxbd[:, hp, :], q_cm[:, hp, :],
                                    start=True, stop=True,
                                )
                                nc.scalar.activation(att_cm[:, hp, :], patt[:], ACTF.Copy)
                            # Wr + residual -> f3out (+ LN3 stats via bn_stats)
                            for t in range(2):
                                po = ps_mm.tile([128, 512], F32, tag="mmq", bufs=3)
                                nc.tensor.matmul(po[:], ones_sb[:], brrow_sb[:], start=True, stop=False)
                                for cc in range(2):
                                    nc.tensor.matmul(
                                        po[:],
                                        att_cm[:, 2 * cc:2 * cc + 2, t * 128:(t + 1) * 128],
                                        wr_sb[:, 2 * cc:2 * cc + 2, :],
                                        start=False, stop=(cc == 1),
                                        perf_mode=DR,
                                    )
                                f3o = p13.tile([128, DIM], BF16, tag="f3o")
                                nc.vector.scalar_tensor_tensor(
                                    f3o[:], po[:], DESCALE, x3[:, t, :],
                                    op0=ALU.mult, op1=ALU.add,
                                )
                                bns3 = p1.tile([128, 6], F32, tag="bns3")
                                nc.vector.bn_stats(bns3[:], f3o[:])
                                nc.vector.bn_aggr(stats3[:, 2 * s + t, :], bns3[:])
                                nc.sync.dma_start(
                                    f3o_dram[s, t * 128:(t + 1) * 128, :], f3o[:]
                                )

                # ================= STAGE 2 =================
                with (
                    tc.tile_pool(name="s2_sb", bufs=3) as p2,
                    tc.tile_pool(name="s2_sb3", bufs=3) as p23,
                    tc.tile_pool(name="ps2_w1", bufs=2, space="PSUM") as ps2_w1,
                    tc.tile_pool(name="ps2_w2", bufs=4, space="PSUM") as ps2_w2,
                ):
                    # LN3 stats math for all samples at once
                    negm3 = p2.tile([128, 2 * NS], F32, tag="negm3")
                    nc.vector.tensor_scalar_mul(negm3[:], stats3[:, :, 0], -1.0)
                    lnv3 = p2.tile([128, 2 * NS], F32, tag="lnv3")
                    nc.scalar.activation(lnv3[:], stats3[:, :, 1], ACTF.Ln, bias=eps_sb[:])
                    s3 = p2.tile([128, 2 * NS], F32, tag="s3")
                    nc.scalar.activation(s3[:], lnv3[:], ACTF.Exp, scale=-0.5)
                    negms3 = p2.tile([128, 2 * NS], F32, tag="negms3")
                    nc.vector.tensor_tensor(negms3[:], negm3[:], s3[:], op=ALU.mult)

                    for g in range(NG2):
                        f3o2 = p23.tile([128, 4, DIM], BF16, tag="f3o2")
                        for c in range(4):
                            nc.sync.dma_start(
                                f3o2[:, c, :],
                                f3o_dram[2 * g + c // 2, (c % 2) * 128:(c % 2) * 128 + 128, :],
                            )
                        # LN3 apply on the scalar engine: x*s3 + (-m*s3)
                        xoh = p2.tile([128, 4, DIM], BF16, tag="xoh")
                        for c in range(4):
                            col = 4 * g + c
                            nc.scalar.activation(
                                xoh[:, c, :], f3o2[:, c, :], ACTF.Identity,
                                bias=negms3[:, col: col + 1],
                                scale=s3[:, col: col + 1],
                            )
                        xoc_bf = p2.tile([128, 4, DIM], BF16, tag="xoc_bf")
                        for c in range(4):
                            nc.sync.dma_start_transpose(
                                xoc_bf[:, :, c * 128:(c + 1) * 128], xoh[:, c, :]
                            )
                        xoc = p2.tile([128, 4, DIM], FP8, tag="xoc")
                        for h in range(2):
                            nc.vector.tensor_copy(
                                xoc[:, 2 * h:2 * h + 2, :], xoc_bf[:, 2 * h:2 * h + 2, :]
                            )
                        pf = []
                        for c in range(4):
                            pfc = ps2_w2.tile([128, 512], F32, tag="w2acc")
                            nc.tensor.matmul(pfc[:], ones_sb[:], b2row_sb[:], start=True, stop=False)
                            pf.append(pfc)
                        for hp in range(8):
                            gt2 = p23.tile([128, 2, DIM], FP8, tag="gt")
                            for i in range(2):
                                hc = 2 * hp + i
                                pw1 = ps2_w1.tile([128, 512], F32, tag="w1ps")
                                for kc in range(2):
                                    nc.tensor.matmul(
                                        pw1[:],
                                        w1_sb[:, 2 * kc:2 * kc + 2, hc * 128:(hc + 1) * 128],
                                        xoc[:, 2 * kc:2 * kc + 2, :],
                                        start=(kc == 0), stop=(kc == 1),
                                        perf_mode=DR,
                                    )
                                nc.scalar.activation(
                                    gt2[:, i, :], pw1[:], ACTF.Gelu,
                                    bias=b1_sb[:, hc: hc + 1], scale=DESCALE,
                                )
                            for c in range(4):
                                nc.tensor.matmul(
                                    pf[c][:],
                                    gt2[:, :, c * 128:(c + 1) * 128],
                                    w2_sb[:, 2 * hp:2 * hp + 2, :],
                                    start=False, stop=(hp == 7),
                                    perf_mode=DR,
                                    skip_group_check=True,
                                )
                        for c in range(4):
                            outt = p2.tile([128, DIM], F32, tag="outt")
                            nc.vector.scalar_tensor_tensor(
                                outt[:], pf[c][:], DESCALE, f3o2[:, c, :],
                                op0=ALU.mult, op1=ALU.add,
                            )
                            nc.sync.dma_start(
                                out[2 * g + c // 2, (c % 2) * 128:(c % 2) * 128 + 128, :],
                                outt[:],
                            )

    nc.compile()
    return nc


def _get_module(n_samples):
    if n_samples not in _BUILD_CACHE:
        _BUILD_CACHE[n_samples] = _build(n_samples)
    return _BUILD_CACHE[n_samples]


def kernel(**inputs) -> np.ndarray:
    from concourse.bass_utils import run_bass_kernel_spmd

    consts = _host_prep(inputs)
    f3 = np.ascontiguousarray(np.asarray(inputs["f3"], dtype=np.float32))
    f4 = np.ascontiguousarray(np.asarray(inputs["f4"], dtype=np.float32))

    nc = _get_module(BSH)
    in_maps = []
    for c in range(N_CORES):
        m = dict(consts)
        m["f3"] = np.ascontiguousarray(f3[c * BSH:(c + 1) * BSH])
        m["f4"] = np.ascontiguousarray(f4[c * BSH:(c + 1) * BSH])
        in_maps.append(m)
    res = run_bass_kernel_spmd(nc, in_maps, core_ids=list(range(N_CORES)))
    return np.concatenate([res.results[c]["out"] for c in range(N_CORES)], axis=0)

